# revision 20
# baseline (speedup 1.0000x reference)
"""Trainium2 Bass kernel for nn_BIOTEncoder_8873402433856.

BIOT encoder: per-sample STFT (as windowed-DFT matmuls) -> patch-frequency
embedding -> 4-layer linear-attention transformer -> (h.mean(axis=1), h).

Sharding: pure data parallel over batch (256 samples) across 8 NeuronCores,
32 samples per core; weights replicated.

Device layout: activations feature-major (E on partitions as 2x128 chunks,
tokens on the free axis). Matmuls run in float32r (rounded fp32, full PE rate
for moving dims >= 256); every matmul operand is produced by a compute
instruction writing float32r. rsqrt/sqrt/reciprocal are computed as
exp(a*ln(x)) to stay inside the single ACT "natural_log_exp" table set; GELU
(exact erf form) is the only other table set, so there are 2 table switches
per layer.
"""

import numpy as np

# problem constants (hardcoded per contract)
B, C, T = 256, 16, 2000
N_FFT, HOP = 200, 100
NF = N_FFT // 2 + 1            # 101
E, D, H = 256, 4, 8
DH = E // H                    # 32
F4 = 4 * E                     # 1024
NFR = (T - N_FFT) // HOP + 1   # 19
N = C * NFR                    # 304 tokens per sample
NCORES = 8
LN_EPS = 1e-5

_CACHE = {}


def _patch_act_tables():
    """Make Ln/Exp resolve only to the combined natural_log_exp set so the
    greedy table-load inserter doesn't thrash between exp_and_others and
    natural_log on every Ln/Exp alternation (set indices preserved)."""
    from concourse import bacc, hw_specs
    import concourse.mybir as mybir
    if getattr(bacc, "_act_tables_patched", False):
        return
    orig = hw_specs.get_activation_tables

    def patched(module_arch):
        t = orig(module_arch)
        AF = mybir.ActivationFunctionType
        for name, fns in t.items():
            if "natural_log_exp" in name:
                continue
            fns.discard(AF.Exp)
            fns.discard(AF.Ln)
        return t

    bacc.get_activation_tables = patched
    bacc._act_tables_patched = True


def _build(nsamp, zero_bres=True):
    import concourse.bass as bass
    from concourse import bacc
    import concourse.mybir as mybir
    import concourse.tile as tile

    _patch_act_tables()

    F32 = mybir.dt.float32
    F32R = mybir.dt.float32r
    AF = mybir.ActivationFunctionType
    ALU = mybir.AluOpType
    AX = mybir.AxisListType

    NT = nsamp * N  # batched token count

    nc = bacc.Bacc("TRN2", target_bir_lowering=False, debug=False, num_devices=1)

    # ---------------- DRAM I/O ----------------
    d_frames = nc.dram_tensor("frames", [nsamp, N_FFT, N], F32, kind="ExternalInput").ap()
    d_gmat = nc.dram_tensor("gmat", [N_FFT, 2 * NF], F32, kind="ExternalInput").ap()
    d_projw = nc.dram_tensor("projw", [NF, E], F32, kind="ExternalInput").ap()
    d_biasT = nc.dram_tensor("biasT", [E, N], F32, kind="ExternalInput").ap()
    d_wq = nc.dram_tensor("wq", [D, E, E], F32, kind="ExternalInput").ap()
    d_wk = nc.dram_tensor("wk", [D, E, E], F32, kind="ExternalInput").ap()
    d_wv = nc.dram_tensor("wv", [D, E, E], F32, kind="ExternalInput").ap()
    d_wo = nc.dram_tensor("wo", [D, E, E], F32, kind="ExternalInput").ap()
    d_w1 = nc.dram_tensor("w1", [D, E, F4], F32, kind="ExternalInput").ap()
    d_w2 = nc.dram_tensor("w2", [D, F4, E], F32, kind="ExternalInput").ap()
    # per-layer per-LN per-partition params, pp layout (128, 2 chunks)
    d_lng = nc.dram_tensor("lng", [D, 2, 128, 2], F32, kind="ExternalInput").ap()
    d_lngn = nc.dram_tensor("lngn", [D, 2, 128, 2], F32, kind="ExternalInput").ap()
    d_bo = nc.dram_tensor("bo", [D, 128, 2], F32, kind="ExternalInput").ap()
    d_b2 = nc.dram_tensor("b2", [D, 128, 2], F32, kind="ExternalInput").ap()
    d_blockones = nc.dram_tensor("blockones", [128, 2, 8], F32, kind="ExternalInput").ap()
    d_expander = nc.dram_tensor("expander", [8, 2, 128], F32, kind="ExternalInput").ap()
    d_sel = nc.dram_tensor("sel", [128, nsamp, nsamp], F32, kind="ExternalInput").ap()
    d_mask = nc.dram_tensor("maskS", [128, 128], F32, kind="ExternalInput").ap()
    d_oh = nc.dram_tensor("out_h", [E, NT], F32, kind="ExternalOutput").ap()
    d_om = nc.dram_tensor("out_mean", [E, nsamp], F32, kind="ExternalOutput").ap()

    def bcast_row(src_row_ap, parts=128):
        # replicate a (1, F) AP across `parts` partitions for a DMA read
        free = [list(d) for d in src_row_ap.ap[1:]]
        return bass.AP(tensor=src_row_ap.tensor, offset=src_row_ap.offset,
                       ap=[[0, parts]] + free)

    def chunk2_bcast(t2d):
        # view a (128, F) tile as (128, 2, F) with a step-0 middle dim
        a = t2d[:, :]
        return bass.AP(tensor=a.tensor, offset=a.offset,
                       ap=[list(a.ap[0]), [0, 2], list(a.ap[1])])

    with tile.TileContext(nc) as tc:
        import contextlib
        with contextlib.ExitStack() as ctx:
            persist = ctx.enter_context(tc.tile_pool(name="persist", bufs=1))
            consts = ctx.enter_context(tc.tile_pool(name="consts", bufs=1))
            wpool = ctx.enter_context(tc.tile_pool(name="wpool", bufs=1))
            wstage = ctx.enter_context(tc.tile_pool(name="wstage", bufs=2))
            stats = ctx.enter_context(tc.tile_pool(name="stats", bufs=2))
            temps = ctx.enter_context(tc.tile_pool(name="temps", bufs=2))
            big1 = ctx.enter_context(tc.tile_pool(name="big1", bufs=1))
            psum = ctx.enter_context(tc.tile_pool(name="psum", bufs=1, space="PSUM"))
            psum2 = ctx.enter_context(tc.tile_pool(name="psum2", bufs=2, space="PSUM"))
            dram = ctx.enter_context(tc.tile_pool(name="dram", bufs=2, space="DRAM"))

            # ---------------- persistent h ----------------
            h = persist.tile([128, 2, NT], F32R)
            mean_sb = persist.tile([128, 2, nsamp], F32)

            # ---------------- constants ----------------
            def load_cast(tag, shape, src_ap, dt=F32R):
                st = wstage.tile(shape, F32, tag="cst_stage")
                nc.sync.dma_start(out=st, in_=src_ap)
                t = consts.tile(shape, dt, tag=tag)
                nc.vector.tensor_copy(t, st)
                return t

            g0_r = load_cast("g0", [128, 2 * NF], d_gmat[0:128, :])
            g1_r = load_cast("g1", [N_FFT - 128, 2 * NF], d_gmat[128:N_FFT, :])
            projw_r = load_cast("projw", [NF, E], d_projw)
            blockones_r = load_cast("blockones", [128, 2, 8], d_blockones)
            expander_r = load_cast("expander", [8, 2, 128], d_expander)
            sel_r = load_cast("sel", [128, nsamp, nsamp], d_sel)
            biasT = consts.tile([128, 2, N], F32)
            nc.sync.dma_start(out=biasT, in_=d_biasT.rearrange("(c p) t -> p c t", p=128))
            maskS = consts.tile([128, 128], F32)
            nc.sync.dma_start(out=maskS, in_=d_mask)
            onescol = consts.tile([128, 1], F32)
            nc.vector.memset(onescol, 1.0)
            eps30 = consts.tile([128, 1], F32)
            nc.vector.memset(eps30, 1e-30)
            epsln = consts.tile([128, 1], F32)
            nc.vector.memset(epsln, LN_EPS)
            onescol_r = consts.tile([128, 1], F32R)
            nc.vector.tensor_copy(onescol_r, onescol)

            # ---------------- prologue: STFT + embedding ----------------
            for s in range(nsamp):
                sl = slice(s * N, (s + 1) * N)
                fA = temps.tile([128, N], F32, tag="fA")
                fB = temps.tile([N_FFT - 128, N], F32, tag="fB")
                nc.sync.dma_start(out=fA, in_=d_frames[s, 0:128, :])
                nc.sync.dma_start(out=fB, in_=d_frames[s, 128:N_FFT, :])
                fAr = temps.tile([128, N], F32R, tag="fAr")
                fBr = temps.tile([N_FFT - 128, N], F32R, tag="fBr")
                nc.gpsimd.tensor_copy(fAr, fA)
                nc.gpsimd.tensor_copy(fBr, fB)
                re_ps = psum2.tile([NF, N], F32, tag="re_ps")
                im_ps = psum2.tile([NF, N], F32, tag="im_ps")
                nc.tensor.matmul(re_ps, g0_r[:, 0:NF], fAr, start=True, stop=False)
                nc.tensor.matmul(re_ps, g1_r[:, 0:NF], fBr, start=False, stop=True)
                nc.tensor.matmul(im_ps, g0_r[:, NF:2 * NF], fAr, start=True, stop=False)
                nc.tensor.matmul(im_ps, g1_r[:, NF:2 * NF], fBr, start=False, stop=True)
                sqre = temps.tile([NF, N], F32, tag="sqre")
                sqim = temps.tile([NF, N], F32, tag="sqim")
                nc.scalar.activation(out=sqre, in_=re_ps, func=AF.Square)
                nc.scalar.activation(out=sqim, in_=im_ps, func=AF.Square)
                ss = temps.tile([NF, N], F32, tag="ss")
                nc.vector.tensor_add(ss, sqre, sqim)
                lnss = temps.tile([NF, N], F32, tag="lnss")
                nc.scalar.activation(out=lnss, in_=ss, func=AF.Ln, bias=eps30[0:NF, :])
                spec_r = temps.tile([NF, N], F32R, tag="spec")
                nc.scalar.activation(out=spec_r, in_=lnss, func=AF.Exp, scale=0.5)
                emb_ps = psum2.tile([128, 2, 512], F32, tag="emb_ps")
                for m in range(2):
                    nc.tensor.matmul(emb_ps[:, m, 0:N], projw_r[:, m * 128:(m + 1) * 128],
                                     spec_r, start=True, stop=True)
                for c_ in range(2):
                    nc.vector.tensor_add(h[:, c_, sl], emb_ps[:, c_, 0:N], biasT[:, c_, :])

            # ---------------- transformer layers ----------------
            for l in range(D):
                # --- load + cast layer weights ---
                def wload(tag, shape, src):
                    st = wstage.tile(shape, F32, tag="wstage")
                    nc.sync.dma_start(out=st, in_=src)
                    t = wpool.tile(shape, F32R, tag=tag)
                    nc.gpsimd.tensor_copy(t, st)
                    return t

                wq_r = wload("wq", [128, 2, E], d_wq[l].rearrange("(kc kp) m -> kp kc m", kp=128))
                wk_r = wload("wk", [128, 2, E], d_wk[l].rearrange("(kc kp) m -> kp kc m", kp=128))
                wv_r = wload("wv", [128, 2, E], d_wv[l].rearrange("(kc kp) m -> kp kc m", kp=128))
                wo_r = wload("wo", [128, 2, E], d_wo[l].rearrange("(kc kp) m -> kp kc m", kp=128))
                w1_r = wload("w1", [128, 2, F4], d_w1[l].rearrange("(kc kp) m -> kp kc m", kp=128))
                w2_r = wload("w2", [128, 8, E], d_w2[l].rearrange("(kc kp) m -> kp kc m", kp=128))
                lnp = consts.tile([128, 2, 2, 2], F32, tag=f"lnp{l}")  # [p, which, g/gneg, c]
                nc.sync.dma_start(out=lnp[:, :, 0, :], in_=d_lng[l].rearrange("w p c -> p w c"))
                nc.sync.dma_start(out=lnp[:, :, 1, :], in_=d_lngn[l].rearrange("w p c -> p w c"))
                bres = consts.tile([128, 2, 2], F32, tag=f"bres{l}")  # [p, which, c]
                nc.sync.dma_start(out=bres[:, 0, :], in_=d_bo[l])
                nc.sync.dma_start(out=bres[:, 1, :], in_=d_b2[l])

                for half in range(2):  # 0: attention, 1: FFN
                    # --- LN stats, two sample-groups so the next phase's early
                    # samples unblock while the late ones still compute ---
                    G = max(1, nsamp // 2)
                    da_g, dmc_g = [], []
                    for grp in range((nsamp + G - 1) // G):
                        lo, hi = grp * G, min(nsamp, (grp + 1) * G)
                        Ssum = ps.tile([nsamp, N], F32, tag="psB", bufs=2)
                        Ssq = ps.tile([nsamp, N], F32, tag="psB", bufs=2)
                        for s in range(lo, hi):
                            sl = slice(s * N, (s + 1) * N)
                            sq = temps.tile([128, 2, N], F32R, tag="sq", bufs=2)
                            nc.vector.tensor_mul(sq, h[:, :, sl].bitcast(F32),
                                                 h[:, :, sl].bitcast(F32))
                            for c_ in range(2):
                                st_ = (s == lo and c_ == 0)
                                sp_ = (s == hi - 1 and c_ == 1)
                                nc.tensor.matmul(Ssum, sel_r[:, s, :], h[:, c_, sl],
                                                 start=st_, stop=sp_)
                                nc.tensor.matmul(Ssq, sel_r[:, s, :], sq[:, c_, :],
                                                 start=st_, stop=sp_)
                        mu = stats.tile([nsamp, N], F32, tag="mu")
                        nc.scalar.activation(out=mu, in_=Ssum, func=AF.Copy, scale=1.0 / E)
                        esq = stats.tile([nsamp, N], F32, tag="esq")
                        nc.scalar.activation(out=esq, in_=Ssq, func=AF.Copy, scale=1.0 / E)
                        mu2 = stats.tile([nsamp, N], F32, tag="mu2")
                        nc.vector.tensor_mul(mu2, mu, mu)
                        var = stats.tile([nsamp, N], F32, tag="var")
                        nc.vector.tensor_sub(var, esq, mu2)
                        lnv = stats.tile([nsamp, N], F32, tag="lnv")
                        nc.scalar.activation(out=lnv, in_=var, func=AF.Ln,
                                             bias=epsln[0:nsamp, :])
                        amc = stats.tile([nsamp, 2, N], F32, tag="amc")
                        nc.scalar.activation(out=amc[:, 0, :], in_=lnv, func=AF.Exp,
                                             scale=-0.5)
                        nc.vector.tensor_mul(amc[:, 1, :], mu, amc[:, 0, :])
                        damc = dram.tile([G, 2, N], F32, tag="damc")
                        nc.sync.dma_start(out=damc[0:hi - lo, :, :], in_=amc[lo:hi, :, :])
                        da_g.append((lo, damc))

                    def damc_row(s):
                        for (lo_, d_) in reversed(da_g):
                            if s >= lo_:
                                return d_[s - lo_:s - lo_ + 1, :, :]

                    for s in range(nsamp):
                        sl = slice(s * N, (s + 1) * N)
                        sq = temps.tile([128, 2, N], F32R, tag="sq", bufs=2)
                        nc.vector.tensor_mul(sq, h[:, :, sl].bitcast(F32),
                                             h[:, :, sl].bitcast(F32))
                        for c_ in range(2):
                            st_ = (s == 0 and c_ == 0)
                            sp_ = (s == nsamp - 1 and c_ == 1)
                            nc.tensor.matmul(Ssum, sel_r[:, s, :], h[:, c_, sl],
                                             start=st_, stop=sp_)
                            nc.tensor.matmul(Ssq, sel_r[:, s, :], sq[:, c_, :],
                                             start=st_, stop=sp_)
                    mu = stats.tile([nsamp, N], F32, tag="mu")
                    nc.scalar.activation(out=mu, in_=Ssum, func=AF.Copy, scale=1.0 / E)
                    esq = stats.tile([nsamp, N], F32, tag="esq")
                    nc.scalar.activation(out=esq, in_=Ssq, func=AF.Copy, scale=1.0 / E)
                    mu2 = stats.tile([nsamp, N], F32, tag="mu2")
                    nc.vector.tensor_mul(mu2, mu, mu)
                    var = stats.tile([nsamp, N], F32, tag="var")
                    nc.vector.tensor_sub(var, esq, mu2)
                    lnv = stats.tile([nsamp, N], F32, tag="lnv")
                    nc.scalar.activation(out=lnv, in_=var, func=AF.Ln, bias=epsln[0:nsamp, :])
                    a_t = stats.tile([nsamp, N], F32, tag="a_t")
                    nc.scalar.activation(out=a_t, in_=lnv, func=AF.Exp, scale=-0.5)
                    mc_t = stats.tile([nsamp, N], F32, tag="mc_t")
                    nc.vector.tensor_mul(mc_t, mu, a_t)
                    da = dram.tile([nsamp, N], F32, tag="da")
                    dmc = dram.tile([nsamp, N], F32, tag="dmc")
                    nc.sync.dma_start(out=da, in_=a_t)
                    nc.sync.dma_start(out=dmc, in_=mc_t)

                    for s in range(nsamp):
                        sl = slice(s * N, (s + 1) * N)
                        amc_b = temps.tile([128, 2, N], F32, tag="amc_b", bufs=5)
                        nc.sync.dma_start(out=amc_b, in_=bcast_row(damc_row(s)))
                        t_ = temps.tile([128, 2, N], F32, tag="t_", bufs=2)
                        y_r = temps.tile([128, 2, N], F32R, tag="y_r", bufs=3)
                        nc.gpsimd.tensor_tensor(
                            out=t_, in0=h[:, :, sl].bitcast(F32),
                            in1=chunk2_bcast(amc_b[:, 0, :]), op=ALU.mult)
                        nc.gpsimd.tensor_tensor(
                            out=y_r, in0=t_, in1=chunk2_bcast(amc_b[:, 1, :]), op=ALU.subtract)

                        if half == 0:
                            # ---------- attention ----------
                            q_ps = psum2.tile([128, 2, 512], F32, tag="q_ps")
                            for m in range(2):
                                for kk in range(2):
                                    nc.tensor.matmul(
                                        q_ps[:, m, 0:N],
                                        wq_r[:, kk, m * 128:(m + 1) * 128],
                                        y_r[:, kk, :], start=(kk == 0), stop=(kk == 1))
                            expq = temps.tile([128, 2, N], F32R, tag="expq", bufs=2)
                            nc.scalar.activation(out=expq, in_=q_ps[:, :, 0:N], func=AF.Exp)
                            Sq_ps = psum2.tile([8, N], F32, tag="Sq_ps")
                            for c_ in range(2):
                                nc.tensor.matmul(Sq_ps, blockones_r[:, c_, :],
                                                 expq[:, c_, :], start=(c_ == 0), stop=(c_ == 1))
                            lnS = temps.tile([8, N], F32, tag="lnS", bufs=2)
                            nc.scalar.activation(out=lnS, in_=Sq_ps, func=AF.Ln)
                            rS_r = temps.tile([8, N], F32R, tag="rS", bufs=2)
                            nc.scalar.activation(out=rS_r, in_=lnS, func=AF.Exp, scale=-1.0)
                            rb_ps = psum2.tile([128, 2, 512], F32, tag="rb_ps")
                            for c_ in range(2):
                                nc.tensor.matmul(rb_ps[:, c_, 0:N], expander_r[:, c_, :],
                                                 rS_r, start=True, stop=True)
                            qn_r = temps.tile([128, 2, N], F32R, tag="qn", bufs=3)
                            nc.vector.tensor_mul(qn_r, expq.bitcast(F32), rb_ps[:, :, 0:N])
                            # kT, vT (token-major)
                            kA_ps = psum2.tile([128, 2, E], F32, tag="kA_ps")
                            kB_ps = psum2.tile([N - 256, E], F32, tag="kB_ps")
                            vA_ps = psum2.tile([128, 2, E], F32, tag="vA_ps")
                            vB_ps = psum2.tile([N - 256, E], F32, tag="vB_ps")
                            for tt in range(3):
                                tsz = 128 if tt < 2 else N - 256
                                tsl = slice(tt * 128, tt * 128 + tsz)
                                for kk in range(2):
                                    st_, sp_ = (kk == 0), (kk == 1)
                                    kdst = kA_ps[:, tt, :] if tt < 2 else kB_ps
                                    vdst = vA_ps[:, tt, :] if tt < 2 else vB_ps
                                    nc.tensor.matmul(kdst[0:tsz, :], y_r[:, kk, tsl],
                                                     wk_r[:, kk, :], start=st_, stop=sp_)
                                    nc.tensor.matmul(vdst[0:tsz, :], y_r[:, kk, tsl],
                                                     wv_r[:, kk, :], start=st_, stop=sp_)
                            ekA_r = temps.tile([128, 2, E], F32R, tag="ekA", bufs=2)
                            ekB_r = temps.tile([N - 256, E], F32R, tag="ekB", bufs=2)
                            nc.scalar.activation(out=ekA_r, in_=kA_ps, func=AF.Exp)
                            nc.scalar.activation(out=ekB_r, in_=kB_ps, func=AF.Exp)
                            vA_r = temps.tile([128, 2, E], F32R, tag="vA", bufs=2)
                            vB_r = temps.tile([N - 256, E], F32R, tag="vB", bufs=2)
                            nc.vector.tensor_copy(vA_r, vA_ps)
                            nc.vector.tensor_copy(vB_r, vB_ps)
                            # Sk (ap=1 matmuls), rk = 1/Sk
                            Sk_ps = psum2.tile([128, 2], F32, tag="Sk_ps")
                            for d_ in range(2):
                                dsl = slice(d_ * 128, (d_ + 1) * 128)
                                for tt in range(3):
                                    tsz = 128 if tt < 2 else N - 256
                                    ek = ekA_r[:, tt, dsl] if tt < 2 else ekB_r[:, dsl]
                                    nc.tensor.matmul(Sk_ps[:, d_:d_ + 1],
                                                     ek[0:tsz, :].bitcast(F32),
                                                     onescol[0:tsz, :],
                                                     start=(tt == 0), stop=(tt == 2))
                            rk = temps.tile([128, 2], F32, tag="rk", bufs=2)
                            nc.vector.reciprocal(rk, Sk_ps)
                            # ctx (block-diagonal useful parts)
                            ctx_ps = psum2.tile([128, 2, E], F32, tag="ctx_ps")
                            for d_ in range(2):
                                dsl = slice(d_ * 128, (d_ + 1) * 128)
                                for tt in range(3):
                                    tsz = 128 if tt < 2 else N - 256
                                    ek = ekA_r[:, tt, dsl] if tt < 2 else ekB_r[:, dsl]
                                    vv = vA_r[:, tt, :] if tt < 2 else vB_r
                                    nc.tensor.matmul(ctx_ps[:, d_, :], ek[0:tsz, :],
                                                     vv[0:tsz, :], start=(tt == 0), stop=(tt == 2))
                            ctxS_r = temps.tile([128, 2, 128], F32R, tag="ctxS", bufs=2)
                            for d_ in range(2):
                                esl = slice(d_ * 128, (d_ + 1) * 128)
                                nc.vector.scalar_tensor_tensor(
                                    out=ctxS_r[:, d_, :], in0=ctx_ps[:, d_, esl],
                                    scalar=rk[:, d_:d_ + 1], in1=maskS,
                                    op0=ALU.mult, op1=ALU.mult)
                            # o = qn @ ctx  (feature-major out)
                            o_ps = psum2.tile([128, 2, 512], F32, tag="o_ps")
                            for c_ in range(2):
                                nc.tensor.matmul(o_ps[:, c_, 0:N], ctxS_r[:, c_, :],
                                                 qn_r[:, c_, :], start=True, stop=True)
                            oS_r = temps.tile([128, 2, N], F32R, tag="oS", bufs=3)
                            nc.vector.tensor_copy(oS_r, o_ps[:, :, 0:N])
                            # Wo projection + residual
                            dh_ps = psum2.tile([128, 2, 512], F32, tag="dh_ps")
                            for m in range(2):
                                for kk in range(2):
                                    nc.tensor.matmul(
                                        dh_ps[:, m, 0:N],
                                        wo_r[:, kk, m * 128:(m + 1) * 128],
                                        oS_r[:, kk, :], start=(kk == 0), stop=(kk == 1))
                            if zero_bres:
                                nc.vector.tensor_add(h[:, :, sl], dh_ps[:, :, 0:N],
                                                     h[:, :, sl].bitcast(F32))
                            else:
                                for c_ in range(2):
                                    nc.vector.scalar_tensor_tensor(
                                        out=h[:, c_, sl], in0=dh_ps[:, c_, 0:N],
                                        scalar=bres[:, 0, c_:c_ + 1], in1=h[:, c_, sl].bitcast(F32),
                                        op0=ALU.add, op1=ALU.add)
                        else:
                            # ---------- FFN ----------
                            midS_r = temps.tile([128, 8, N], F32R, tag="midS", bufs=1)
                            for hb in range(2):
                                mid_ps = psum2.tile([128, 4, 512], F32, tag="mid_ps")
                                for m in range(4):
                                    mg = hb * 4 + m
                                    for kk in range(2):
                                        nc.tensor.matmul(
                                            mid_ps[:, m, 0:N],
                                            w1_r[:, kk, mg * 128:(mg + 1) * 128],
                                            y_r[:, kk, :], start=(kk == 0), stop=(kk == 1))
                                nc.scalar.activation(out=midS_r[:, hb * 4:(hb + 1) * 4, :],
                                                     in_=mid_ps[:, :, 0:N], func=AF.Gelu)
                            dh2_ps = psum2.tile([128, 2, 512], F32, tag="dh2_ps")
                            for m in range(2):
                                for kk in range(8):
                                    nc.tensor.matmul(
                                        dh2_ps[:, m, 0:N],
                                        w2_r[:, kk, m * 128:(m + 1) * 128],
                                        midS_r[:, kk, :], start=(kk == 0), stop=(kk == 7))
                            if zero_bres:
                                nc.vector.tensor_add(h[:, :, sl], dh2_ps[:, :, 0:N],
                                                     h[:, :, sl].bitcast(F32))
                            else:
                                for c_ in range(2):
                                    nc.vector.scalar_tensor_tensor(
                                        out=h[:, c_, sl], in0=dh2_ps[:, c_, 0:N],
                                        scalar=bres[:, 1, c_:c_ + 1], in1=h[:, c_, sl].bitcast(F32),
                                        op0=ALU.add, op1=ALU.add)
                            if l == D - 1:
                                # stream final outputs per sample to overlap the tail
                                red2 = temps.tile([128, 2], F32, tag="red2", bufs=2)
                                nc.vector.tensor_reduce(
                                    out=red2, in_=h[:, :, sl].bitcast(F32),
                                    op=ALU.add, axis=AX.X)
                                nc.vector.tensor_scalar_mul(mean_sb[:, :, s], red2, 1.0 / N)
                                for c_ in range(2):
                                    nc.sync.dma_start(
                                        out=d_oh[c_ * 128:(c_ + 1) * 128, s * N:(s + 1) * N],
                                        in_=h[:, c_, sl].bitcast(F32))

            # ---------------- outputs ----------------
            for c_ in range(2):
                nc.sync.dma_start(out=d_om[c_ * 128:(c_ + 1) * 128, :], in_=mean_sb[:, c_, :])

    nc.compile()
    return nc


def _host_prep(x, n_channel_offset, proj_W, proj_b, chan_tok, Wq, Wk, Wv, Wo, bo,
               ln1_g, ln1_b, ln2_g, ln2_b, W1, b1, W2, b2):
    f32 = np.float32
    x = np.ascontiguousarray(np.asarray(x, f32))
    Bx = x.shape[0]

    # im2col frames, transposed to (B, n_fft, N)
    sb, sc, st = x.strides
    fr = np.lib.stride_tricks.as_strided(
        x, shape=(Bx, C, NFR, N_FFT), strides=(sb, sc, HOP * st, st))
    frames = np.ascontiguousarray(fr.transpose(0, 3, 1, 2).reshape(Bx, N_FFT, N))

    # windowed DFT matrices
    n = np.arange(N_FFT, dtype=np.float64)
    win = 0.5 * (1.0 - np.cos(2.0 * np.pi * n / N_FFT))
    k = np.arange(NF, dtype=np.float64)
    ang = 2.0 * np.pi * np.outer(n, k) / N_FFT
    gmat = np.concatenate([win[:, None] * np.cos(ang),
                           -win[:, None] * np.sin(ang)], axis=1).astype(f32)

    # embedding bias: proj_b + chan_tok slice + positional encoding
    pos = np.arange(NFR, dtype=np.float64)[:, None]
    div = np.exp(np.arange(0, E, 2, dtype=np.float64) * (-np.log(10000.0) / E))
    pe = np.zeros((NFR, E), np.float64)
    pe[:, 0::2] = np.sin(pos * div)
    pe[:, 1::2] = np.cos(pos * div)
    off = int(n_channel_offset)
    ch = np.asarray(chan_tok, f32)[off:off + C]
    bias_all = (np.asarray(proj_b, f32)[None, None, :] + ch[:, None, :]
                + pe[None, :, :].astype(f32)).reshape(N, E)
    biasT = np.ascontiguousarray(bias_all.T)

    pp = lambda v: np.ascontiguousarray(np.asarray(v, f32).reshape(2, 128).T)
    lng = np.stack([np.stack([pp(ln1_g[l]), pp(ln2_g[l])]) for l in range(D)])
    lngn = -lng
    bo_pp = np.stack([pp(np.asarray(bo, f32)[l]) for l in range(D)])
    b2_pp = np.stack([pp(np.asarray(b2, f32)[l]) for l in range(D)])

    # biases that have no device path in this build must be zero
    assert not np.any(np.asarray(b1)), "nonzero b1 not supported by this build"
    assert not np.any(np.asarray(ln1_b)) and not np.any(np.asarray(ln2_b)), \
        "nonzero LN bias not supported by this build"

    blockones = np.zeros((128, 2, 8), f32)
    expander = np.zeros((8, 2, 128), f32)
    for p in range(128):
        blockones[p, 0, p // 32] = 1.0
        blockones[p, 1, 4 + p // 32] = 1.0
        expander[p // 32, 0, p] = 1.0
        expander[4 + p // 32, 1, p] = 1.0
    maskS = np.zeros((128, 128), f32)
    sc_ = 1.0 / np.sqrt(DH)
    for d_ in range(128):
        blk = d_ // 32
        maskS[d_, blk * 32:(blk + 1) * 32] = sc_

    g1v = np.asarray(ln1_g, f32)[:, :, None]   # (D, E, 1)
    g2v = np.asarray(ln2_g, f32)[:, :, None]
    com = dict(
        gmat=gmat, projw=np.asarray(proj_W, f32), biasT=biasT,
        wq=np.asarray(Wq, f32) * g1v, wk=np.asarray(Wk, f32) * g1v,
        wv=np.asarray(Wv, f32) * g1v,
        wo=np.asarray(Wo, f32), w1=np.asarray(W1, f32) * g2v, w2=np.asarray(W2, f32),
        lng=lng, lngn=lngn, bo=bo_pp, b2=b2_pp,
        blockones=blockones, expander=expander, maskS=maskS)
    return frames, com


def kernel(**inputs):
    from concourse.bass_utils import run_bass_kernel_spmd

    frames, com = _host_prep(**inputs)
    Bx = frames.shape[0]
    nsamp = Bx // NCORES
    sel = np.zeros((128, nsamp, nsamp), np.float32)
    for s in range(nsamp):
        sel[:, s, s] = 1.0
    com["sel"] = sel

    zb = (not np.any(np.asarray(inputs["bo"]))) and (not np.any(np.asarray(inputs["b2"])))
    key = (nsamp, zb)
    if key not in _CACHE:
        _CACHE[key] = _build(nsamp, zero_bres=zb)
    nc = _CACHE[key]

    in_maps = []
    for c in range(NCORES):
        m = dict(com)
        m["frames"] = np.ascontiguousarray(frames[c * nsamp:(c + 1) * nsamp])
        in_maps.append(m)
    res = run_bass_kernel_spmd(nc, in_maps, list(range(NCORES))).results

    h_out = np.empty((Bx, N, E), np.float32)
    mean_out = np.empty((Bx, E), np.float32)
    for c in range(NCORES):
        oh = res[c]["out_h"]                       # (E, nsamp*N)
        h_out[c * nsamp:(c + 1) * nsamp] = (
            oh.reshape(E, nsamp, N).transpose(1, 2, 0))
        mean_out[c * nsamp:(c + 1) * nsamp] = res[c]["out_mean"].T
    return mean_out, h_out


# revision 22
# speedup vs baseline: 1.0073x; 1.0073x over previous
"""Trainium2 Bass kernel for nn_BIOTEncoder_8873402433856.

BIOT encoder: per-sample STFT (as windowed-DFT matmuls) -> patch-frequency
embedding -> 4-layer linear-attention transformer -> (h.mean(axis=1), h).

Sharding: pure data parallel over batch (256 samples) across 8 NeuronCores,
32 samples per core; weights replicated.

Device layout: activations feature-major (E on partitions as 2x128 chunks,
tokens on the free axis). Matmuls run in float32r (rounded fp32, full PE rate
for moving dims >= 256); every matmul operand is produced by a compute
instruction writing float32r. rsqrt/sqrt/reciprocal are computed as
exp(a*ln(x)) to stay inside the single ACT "natural_log_exp" table set; GELU
(exact erf form) is the only other table set, so there are 2 table switches
per layer.
"""

import numpy as np

# problem constants (hardcoded per contract)
B, C, T = 256, 16, 2000
N_FFT, HOP = 200, 100
NF = N_FFT // 2 + 1            # 101
E, D, H = 256, 4, 8
DH = E // H                    # 32
F4 = 4 * E                     # 1024
NFR = (T - N_FFT) // HOP + 1   # 19
N = C * NFR                    # 304 tokens per sample
NCORES = 8
LN_EPS = 1e-5

_CACHE = {}


def _patch_act_tables():
    """Make Ln/Exp resolve only to the combined natural_log_exp set so the
    greedy table-load inserter doesn't thrash between exp_and_others and
    natural_log on every Ln/Exp alternation (set indices preserved)."""
    from concourse import bacc, hw_specs
    import concourse.mybir as mybir
    if getattr(bacc, "_act_tables_patched", False):
        return
    orig = hw_specs.get_activation_tables

    def patched(module_arch):
        t = orig(module_arch)
        AF = mybir.ActivationFunctionType
        for name, fns in t.items():
            if "natural_log_exp" in name:
                continue
            fns.discard(AF.Exp)
            fns.discard(AF.Ln)
        return t

    bacc.get_activation_tables = patched
    bacc._act_tables_patched = True


def _build(nsamp, zero_bres=True):
    import concourse.bass as bass
    from concourse import bacc
    import concourse.mybir as mybir
    import concourse.tile as tile

    _patch_act_tables()

    F32 = mybir.dt.float32
    F32R = mybir.dt.float32r
    AF = mybir.ActivationFunctionType
    ALU = mybir.AluOpType
    AX = mybir.AxisListType

    NT = nsamp * N  # batched token count

    nc = bacc.Bacc("TRN2", target_bir_lowering=False, debug=False, num_devices=1)

    # ---------------- DRAM I/O ----------------
    d_frames = nc.dram_tensor("frames", [nsamp, N_FFT, N], F32, kind="ExternalInput").ap()
    d_gmat = nc.dram_tensor("gmat", [N_FFT, 2 * NF], F32, kind="ExternalInput").ap()
    d_projw = nc.dram_tensor("projw", [NF, E], F32, kind="ExternalInput").ap()
    d_biasT = nc.dram_tensor("biasT", [E, N], F32, kind="ExternalInput").ap()
    d_wq = nc.dram_tensor("wq", [D, E, E], F32, kind="ExternalInput").ap()
    d_wk = nc.dram_tensor("wk", [D, E, E], F32, kind="ExternalInput").ap()
    d_wv = nc.dram_tensor("wv", [D, E, E], F32, kind="ExternalInput").ap()
    d_wo = nc.dram_tensor("wo", [D, E, E], F32, kind="ExternalInput").ap()
    d_w1 = nc.dram_tensor("w1", [D, E, F4], F32, kind="ExternalInput").ap()
    d_w2 = nc.dram_tensor("w2", [D, F4, E], F32, kind="ExternalInput").ap()
    # per-layer per-LN per-partition params, pp layout (128, 2 chunks)
    d_lng = nc.dram_tensor("lng", [D, 2, 128, 2], F32, kind="ExternalInput").ap()
    d_lngn = nc.dram_tensor("lngn", [D, 2, 128, 2], F32, kind="ExternalInput").ap()
    d_bo = nc.dram_tensor("bo", [D, 128, 2], F32, kind="ExternalInput").ap()
    d_b2 = nc.dram_tensor("b2", [D, 128, 2], F32, kind="ExternalInput").ap()
    d_blockones = nc.dram_tensor("blockones", [128, 2, 8], F32, kind="ExternalInput").ap()
    d_expander = nc.dram_tensor("expander", [8, 2, 128], F32, kind="ExternalInput").ap()
    d_sel = nc.dram_tensor("sel", [128, nsamp, nsamp], F32, kind="ExternalInput").ap()
    d_mask = nc.dram_tensor("maskS", [128, 128], F32, kind="ExternalInput").ap()
    d_oh = nc.dram_tensor("out_h", [E, NT], F32, kind="ExternalOutput").ap()
    d_om = nc.dram_tensor("out_mean", [E, nsamp], F32, kind="ExternalOutput").ap()

    def bcast_row(src_row_ap, parts=128):
        # replicate a (1, F) AP across `parts` partitions for a DMA read
        free = [list(d) for d in src_row_ap.ap[1:]]
        return bass.AP(tensor=src_row_ap.tensor, offset=src_row_ap.offset,
                       ap=[[0, parts]] + free)

    def chunk2_bcast(t2d):
        # view a (128, F) tile as (128, 2, F) with a step-0 middle dim
        a = t2d[:, :]
        return bass.AP(tensor=a.tensor, offset=a.offset,
                       ap=[list(a.ap[0]), [0, 2], list(a.ap[1])])

    with tile.TileContext(nc) as tc:
        import contextlib
        with contextlib.ExitStack() as ctx:
            persist = ctx.enter_context(tc.tile_pool(name="persist", bufs=1))
            consts = ctx.enter_context(tc.tile_pool(name="consts", bufs=1))
            wpool = ctx.enter_context(tc.tile_pool(name="wpool", bufs=1))
            wstage = ctx.enter_context(tc.tile_pool(name="wstage", bufs=2))
            stats = ctx.enter_context(tc.tile_pool(name="stats", bufs=2))
            temps = ctx.enter_context(tc.tile_pool(name="temps", bufs=2))
            big1 = ctx.enter_context(tc.tile_pool(name="big1", bufs=1))
            psum = ctx.enter_context(tc.tile_pool(name="psum", bufs=1, space="PSUM"))
            psum2 = ctx.enter_context(tc.tile_pool(name="psum2", bufs=2, space="PSUM"))
            dram = ctx.enter_context(tc.tile_pool(name="dram", bufs=2, space="DRAM"))

            # ---------------- persistent h ----------------
            h = persist.tile([128, 2, NT], F32R)
            mean_sb = persist.tile([128, 2, nsamp], F32)

            # ---------------- constants ----------------
            def load_cast(tag, shape, src_ap, dt=F32R):
                st = wstage.tile(shape, F32, tag="cst_stage")
                nc.sync.dma_start(out=st, in_=src_ap)
                t = consts.tile(shape, dt, tag=tag)
                nc.vector.tensor_copy(t, st)
                return t

            g0_r = load_cast("g0", [128, 2 * NF], d_gmat[0:128, :])
            g1_r = load_cast("g1", [N_FFT - 128, 2 * NF], d_gmat[128:N_FFT, :])
            projw_r = load_cast("projw", [NF, E], d_projw)
            blockones_r = load_cast("blockones", [128, 2, 8], d_blockones)
            expander_r = load_cast("expander", [8, 2, 128], d_expander)
            sel_r = load_cast("sel", [128, nsamp, nsamp], d_sel)
            biasT = consts.tile([128, 2, N], F32)
            nc.sync.dma_start(out=biasT, in_=d_biasT.rearrange("(c p) t -> p c t", p=128))
            maskS = consts.tile([128, 128], F32)
            nc.sync.dma_start(out=maskS, in_=d_mask)
            onescol = consts.tile([128, 1], F32)
            nc.vector.memset(onescol, 1.0)
            eps30 = consts.tile([128, 1], F32)
            nc.vector.memset(eps30, 1e-30)
            epsln = consts.tile([128, 1], F32)
            nc.vector.memset(epsln, LN_EPS)
            onescol_r = consts.tile([128, 1], F32R)
            nc.vector.tensor_copy(onescol_r, onescol)

            # ---------------- prologue: STFT + embedding ----------------
            for s in range(nsamp):
                sl = slice(s * N, (s + 1) * N)
                fA = temps.tile([128, N], F32, tag="fA")
                fB = temps.tile([N_FFT - 128, N], F32, tag="fB")
                nc.sync.dma_start(out=fA, in_=d_frames[s, 0:128, :])
                nc.sync.dma_start(out=fB, in_=d_frames[s, 128:N_FFT, :])
                fAr = temps.tile([128, N], F32R, tag="fAr")
                fBr = temps.tile([N_FFT - 128, N], F32R, tag="fBr")
                nc.gpsimd.tensor_copy(fAr, fA)
                nc.gpsimd.tensor_copy(fBr, fB)
                re_ps = psum2.tile([NF, N], F32, tag="re_ps")
                im_ps = psum2.tile([NF, N], F32, tag="im_ps")
                nc.tensor.matmul(re_ps, g0_r[:, 0:NF], fAr, start=True, stop=False)
                nc.tensor.matmul(re_ps, g1_r[:, 0:NF], fBr, start=False, stop=True)
                nc.tensor.matmul(im_ps, g0_r[:, NF:2 * NF], fAr, start=True, stop=False)
                nc.tensor.matmul(im_ps, g1_r[:, NF:2 * NF], fBr, start=False, stop=True)
                sqre = temps.tile([NF, N], F32, tag="sqre")
                sqim = temps.tile([NF, N], F32, tag="sqim")
                nc.scalar.activation(out=sqre, in_=re_ps, func=AF.Square)
                nc.scalar.activation(out=sqim, in_=im_ps, func=AF.Square)
                ss = temps.tile([NF, N], F32, tag="ss")
                nc.vector.tensor_add(ss, sqre, sqim)
                lnss = temps.tile([NF, N], F32, tag="lnss")
                nc.scalar.activation(out=lnss, in_=ss, func=AF.Ln, bias=eps30[0:NF, :])
                spec_r = temps.tile([NF, N], F32R, tag="spec")
                nc.scalar.activation(out=spec_r, in_=lnss, func=AF.Exp, scale=0.5)
                emb_ps = psum2.tile([128, 2, 512], F32, tag="emb_ps")
                for m in range(2):
                    nc.tensor.matmul(emb_ps[:, m, 0:N], projw_r[:, m * 128:(m + 1) * 128],
                                     spec_r, start=True, stop=True)
                for c_ in range(2):
                    nc.vector.tensor_add(h[:, c_, sl], emb_ps[:, c_, 0:N], biasT[:, c_, :])

            # ---------------- transformer layers ----------------
            for l in range(D):
                # --- load + cast layer weights ---
                def wload(tag, shape, src):
                    st = wstage.tile(shape, F32, tag="wstage")
                    nc.sync.dma_start(out=st, in_=src)
                    t = wpool.tile(shape, F32R, tag=tag)
                    nc.gpsimd.tensor_copy(t, st)
                    return t

                wq_r = wload("wq", [128, 2, E], d_wq[l].rearrange("(kc kp) m -> kp kc m", kp=128))
                wk_r = wload("wk", [128, 2, E], d_wk[l].rearrange("(kc kp) m -> kp kc m", kp=128))
                wv_r = wload("wv", [128, 2, E], d_wv[l].rearrange("(kc kp) m -> kp kc m", kp=128))
                wo_r = wload("wo", [128, 2, E], d_wo[l].rearrange("(kc kp) m -> kp kc m", kp=128))
                w1_r = wload("w1", [128, 2, F4], d_w1[l].rearrange("(kc kp) m -> kp kc m", kp=128))
                w2_r = wload("w2", [128, 8, E], d_w2[l].rearrange("(kc kp) m -> kp kc m", kp=128))
                lnp = consts.tile([128, 2, 2, 2], F32, tag=f"lnp{l}")  # [p, which, g/gneg, c]
                nc.sync.dma_start(out=lnp[:, :, 0, :], in_=d_lng[l].rearrange("w p c -> p w c"))
                nc.sync.dma_start(out=lnp[:, :, 1, :], in_=d_lngn[l].rearrange("w p c -> p w c"))
                bres = consts.tile([128, 2, 2], F32, tag=f"bres{l}")  # [p, which, c]
                nc.sync.dma_start(out=bres[:, 0, :], in_=d_bo[l])
                nc.sync.dma_start(out=bres[:, 1, :], in_=d_b2[l])

                for half in range(2):  # 0: attention, 1: FFN
                    # --- LN stats, two sample-groups so the next phase's early
                    # samples unblock while the late ones still compute ---
                    G = max(1, nsamp // 2)
                    da_g, dmc_g = [], []
                    for grp in range((nsamp + G - 1) // G):
                        lo, hi = grp * G, min(nsamp, (grp + 1) * G)
                        Ssum = ps.tile([nsamp, N], F32, tag="psB", bufs=2)
                        Ssq = ps.tile([nsamp, N], F32, tag="psB", bufs=2)
                        for s in range(lo, hi):
                            sl = slice(s * N, (s + 1) * N)
                            sq = temps.tile([128, 2, N], F32R, tag="sq", bufs=2)
                            nc.vector.tensor_mul(sq, h[:, :, sl].bitcast(F32),
                                                 h[:, :, sl].bitcast(F32))
                            for c_ in range(2):
                                st_ = (s == lo and c_ == 0)
                                sp_ = (s == hi - 1 and c_ == 1)
                                nc.tensor.matmul(Ssum, sel_r[:, s, :], h[:, c_, sl],
                                                 start=st_, stop=sp_)
                                nc.tensor.matmul(Ssq, sel_r[:, s, :], sq[:, c_, :],
                                                 start=st_, stop=sp_)
                        mu = stats.tile([nsamp, N], F32, tag="mu")
                        nc.scalar.activation(out=mu, in_=Ssum, func=AF.Copy, scale=1.0 / E)
                        esq = stats.tile([nsamp, N], F32, tag="esq")
                        nc.scalar.activation(out=esq, in_=Ssq, func=AF.Copy, scale=1.0 / E)
                        mu2 = stats.tile([nsamp, N], F32, tag="mu2")
                        nc.vector.tensor_mul(mu2, mu, mu)
                        var = stats.tile([nsamp, N], F32, tag="var")
                        nc.vector.tensor_sub(var, esq, mu2)
                        lnv = stats.tile([nsamp, N], F32, tag="lnv")
                        nc.scalar.activation(out=lnv, in_=var, func=AF.Ln,
                                             bias=epsln[0:nsamp, :])
                        amc = stats.tile([nsamp, 2, N], F32, tag="amc")
                        nc.scalar.activation(out=amc[:, 0, :], in_=lnv, func=AF.Exp,
                                             scale=-0.5)
                        nc.vector.tensor_mul(amc[:, 1, :], mu, amc[:, 0, :])
                        damc = dram.tile([G, 2, N], F32, tag="damc")
                        nc.sync.dma_start(out=damc[0:hi - lo, :, :], in_=amc[lo:hi, :, :])
                        da_g.append((lo, damc))

                    def damc_row(s):
                        for (lo_, d_) in reversed(da_g):
                            if s >= lo_:
                                return d_[s - lo_:s - lo_ + 1, :, :]

                    for s in range(nsamp):
                        sl = slice(s * N, (s + 1) * N)
                        sq = temps.tile([128, 2, N], F32R, tag="sq", bufs=2)
                        nc.vector.tensor_mul(sq, h[:, :, sl].bitcast(F32),
                                             h[:, :, sl].bitcast(F32))
                        for c_ in range(2):
                            st_ = (s == 0 and c_ == 0)
                            sp_ = (s == nsamp - 1 and c_ == 1)
                            nc.tensor.matmul(Ssum, sel_r[:, s, :], h[:, c_, sl],
                                             start=st_, stop=sp_)
                            nc.tensor.matmul(Ssq, sel_r[:, s, :], sq[:, c_, :],
                                             start=st_, stop=sp_)
                    mu = stats.tile([nsamp, N], F32, tag="mu")
                    nc.scalar.activation(out=mu, in_=Ssum, func=AF.Copy, scale=1.0 / E)
                    esq = stats.tile([nsamp, N], F32, tag="esq")
                    nc.scalar.activation(out=esq, in_=Ssq, func=AF.Copy, scale=1.0 / E)
                    mu2 = stats.tile([nsamp, N], F32, tag="mu2")
                    nc.vector.tensor_mul(mu2, mu, mu)
                    var = stats.tile([nsamp, N], F32, tag="var")
                    nc.vector.tensor_sub(var, esq, mu2)
                    lnv = stats.tile([nsamp, N], F32, tag="lnv")
                    nc.scalar.activation(out=lnv, in_=var, func=AF.Ln, bias=epsln[0:nsamp, :])
                    a_t = stats.tile([nsamp, N], F32, tag="a_t")
                    nc.scalar.activation(out=a_t, in_=lnv, func=AF.Exp, scale=-0.5)
                    mc_t = stats.tile([nsamp, N], F32, tag="mc_t")
                    nc.vector.tensor_mul(mc_t, mu, a_t)
                    da = dram.tile([nsamp, N], F32, tag="da")
                    dmc = dram.tile([nsamp, N], F32, tag="dmc")
                    nc.sync.dma_start(out=da, in_=a_t)
                    nc.sync.dma_start(out=dmc, in_=mc_t)

                    for s in range(nsamp):
                        sl = slice(s * N, (s + 1) * N)
                        amc_b = temps.tile([128, 2, N], F32, tag="amc_b", bufs=5)
                        nc.sync.dma_start(out=amc_b, in_=bcast_row(damc_row(s)))
                        t_ = temps.tile([128, 2, N], F32, tag="t_", bufs=2)
                        y_r = temps.tile([128, 2, N], F32R, tag="y_r", bufs=3)
                        nc.gpsimd.tensor_tensor(
                            out=t_, in0=h[:, :, sl].bitcast(F32),
                            in1=chunk2_bcast(amc_b[:, 0, :]), op=ALU.mult)
                        nc.gpsimd.tensor_tensor(
                            out=y_r, in0=t_, in1=chunk2_bcast(amc_b[:, 1, :]), op=ALU.subtract)

                        if half == 0:
                            # ---------- attention ----------
                            q_ps = psum2.tile([128, 2, 512], F32, tag="q_ps")
                            for m in range(2):
                                for kk in range(2):
                                    nc.tensor.matmul(
                                        q_ps[:, m, 0:N],
                                        wq_r[:, kk, m * 128:(m + 1) * 128],
                                        y_r[:, kk, :], start=(kk == 0), stop=(kk == 1))
                            expq = temps.tile([128, 2, N], F32R, tag="expq", bufs=2)
                            nc.scalar.activation(out=expq, in_=q_ps[:, :, 0:N], func=AF.Exp)
                            Sq_ps = psum2.tile([8, N], F32, tag="Sq_ps")
                            for c_ in range(2):
                                nc.tensor.matmul(Sq_ps, blockones_r[:, c_, :],
                                                 expq[:, c_, :], start=(c_ == 0), stop=(c_ == 1))
                            lnS = temps.tile([8, N], F32, tag="lnS", bufs=2)
                            nc.scalar.activation(out=lnS, in_=Sq_ps, func=AF.Ln)
                            rS_r = temps.tile([8, N], F32R, tag="rS", bufs=2)
                            nc.scalar.activation(out=rS_r, in_=lnS, func=AF.Exp, scale=-1.0)
                            rb_ps = psum2.tile([128, 2, 512], F32, tag="rb_ps")
                            for c_ in range(2):
                                nc.tensor.matmul(rb_ps[:, c_, 0:N], expander_r[:, c_, :],
                                                 rS_r, start=True, stop=True)
                            qn_r = temps.tile([128, 2, N], F32R, tag="qn", bufs=3)
                            nc.vector.tensor_mul(qn_r, expq.bitcast(F32), rb_ps[:, :, 0:N])
                            # kT, vT (token-major)
                            kA_ps = psum2.tile([128, 2, E], F32, tag="kA_ps")
                            kB_ps = psum2.tile([N - 256, E], F32, tag="kB_ps")
                            vA_ps = psum2.tile([128, 2, E], F32, tag="vA_ps")
                            vB_ps = psum2.tile([N - 256, E], F32, tag="vB_ps")
                            for tt in range(3):
                                tsz = 128 if tt < 2 else N - 256
                                tsl = slice(tt * 128, tt * 128 + tsz)
                                for kk in range(2):
                                    st_, sp_ = (kk == 0), (kk == 1)
                                    kdst = kA_ps[:, tt, :] if tt < 2 else kB_ps
                                    vdst = vA_ps[:, tt, :] if tt < 2 else vB_ps
                                    nc.tensor.matmul(kdst[0:tsz, :], y_r[:, kk, tsl],
                                                     wk_r[:, kk, :], start=st_, stop=sp_)
                                    nc.tensor.matmul(vdst[0:tsz, :], y_r[:, kk, tsl],
                                                     wv_r[:, kk, :], start=st_, stop=sp_)
                            ekA_r = temps.tile([128, 2, E], F32R, tag="ekA", bufs=2)
                            ekB_r = temps.tile([N - 256, E], F32R, tag="ekB", bufs=2)
                            nc.scalar.activation(out=ekA_r, in_=kA_ps, func=AF.Exp)
                            nc.scalar.activation(out=ekB_r, in_=kB_ps, func=AF.Exp)
                            vA_r = temps.tile([128, 2, E], F32R, tag="vA", bufs=2)
                            vB_r = temps.tile([N - 256, E], F32R, tag="vB", bufs=2)
                            nc.vector.tensor_copy(vA_r, vA_ps)
                            nc.vector.tensor_copy(vB_r, vB_ps)
                            # Sk (ap=1 matmuls), rk = 1/Sk
                            Sk_ps = psum2.tile([128, 2], F32, tag="Sk_ps")
                            for d_ in range(2):
                                dsl = slice(d_ * 128, (d_ + 1) * 128)
                                for tt in range(3):
                                    tsz = 128 if tt < 2 else N - 256
                                    ek = ekA_r[:, tt, dsl] if tt < 2 else ekB_r[:, dsl]
                                    nc.tensor.matmul(Sk_ps[:, d_:d_ + 1],
                                                     ek[0:tsz, :].bitcast(F32),
                                                     onescol[0:tsz, :],
                                                     start=(tt == 0), stop=(tt == 2))
                            rk = temps.tile([128, 2], F32, tag="rk", bufs=2)
                            nc.vector.reciprocal(rk, Sk_ps)
                            # ctx (block-diagonal useful parts)
                            ctx_ps = psum2.tile([128, 2, E], F32, tag="ctx_ps")
                            for d_ in range(2):
                                dsl = slice(d_ * 128, (d_ + 1) * 128)
                                for tt in range(3):
                                    tsz = 128 if tt < 2 else N - 256
                                    ek = ekA_r[:, tt, dsl] if tt < 2 else ekB_r[:, dsl]
                                    vv = vA_r[:, tt, :] if tt < 2 else vB_r
                                    nc.tensor.matmul(ctx_ps[:, d_, :], ek[0:tsz, :],
                                                     vv[0:tsz, :], start=(tt == 0), stop=(tt == 2))
                            ctxS_r = temps.tile([128, 2, 128], F32R, tag="ctxS", bufs=2)
                            for d_ in range(2):
                                esl = slice(d_ * 128, (d_ + 1) * 128)
                                nc.vector.scalar_tensor_tensor(
                                    out=ctxS_r[:, d_, :], in0=ctx_ps[:, d_, esl],
                                    scalar=rk[:, d_:d_ + 1], in1=maskS,
                                    op0=ALU.mult, op1=ALU.mult)
                            # o = qn @ ctx  (feature-major out)
                            o_ps = psum2.tile([128, 2, 512], F32, tag="o_ps")
                            for c_ in range(2):
                                nc.tensor.matmul(o_ps[:, c_, 0:N], ctxS_r[:, c_, :],
                                                 qn_r[:, c_, :], start=True, stop=True)
                            oS_r = temps.tile([128, 2, N], F32R, tag="oS", bufs=3)
                            nc.vector.tensor_copy(oS_r, o_ps[:, :, 0:N])
                            # Wo projection + residual
                            dh_ps = psum2.tile([128, 2, 512], F32, tag="dh_ps")
                            for m in range(2):
                                for kk in range(2):
                                    nc.tensor.matmul(
                                        dh_ps[:, m, 0:N],
                                        wo_r[:, kk, m * 128:(m + 1) * 128],
                                        oS_r[:, kk, :], start=(kk == 0), stop=(kk == 1))
                            if zero_bres:
                                nc.vector.tensor_add(h[:, :, sl], dh_ps[:, :, 0:N],
                                                     h[:, :, sl].bitcast(F32))
                            else:
                                for c_ in range(2):
                                    nc.vector.scalar_tensor_tensor(
                                        out=h[:, c_, sl], in0=dh_ps[:, c_, 0:N],
                                        scalar=bres[:, 0, c_:c_ + 1], in1=h[:, c_, sl].bitcast(F32),
                                        op0=ALU.add, op1=ALU.add)
                        else:
                            # ---------- FFN ----------
                            midS_r = temps.tile([128, 8, N], F32R, tag="midS", bufs=1)
                            for hb in range(2):
                                mid_ps = psum2.tile([128, 4, 512], F32, tag="mid_ps")
                                for m in range(4):
                                    mg = hb * 4 + m
                                    for kk in range(2):
                                        nc.tensor.matmul(
                                            mid_ps[:, m, 0:N],
                                            w1_r[:, kk, mg * 128:(mg + 1) * 128],
                                            y_r[:, kk, :], start=(kk == 0), stop=(kk == 1))
                                nc.scalar.activation(out=midS_r[:, hb * 4:(hb + 1) * 4, :],
                                                     in_=mid_ps[:, :, 0:N], func=AF.Gelu)
                            dh2_ps = psum2.tile([128, 2, 512], F32, tag="dh2_ps")
                            for m in range(2):
                                for kk in range(8):
                                    nc.tensor.matmul(
                                        dh2_ps[:, m, 0:N],
                                        w2_r[:, kk, m * 128:(m + 1) * 128],
                                        midS_r[:, kk, :], start=(kk == 0), stop=(kk == 7))
                            if zero_bres:
                                nc.vector.tensor_add(h[:, :, sl], dh2_ps[:, :, 0:N],
                                                     h[:, :, sl].bitcast(F32))
                            else:
                                for c_ in range(2):
                                    nc.vector.scalar_tensor_tensor(
                                        out=h[:, c_, sl], in0=dh2_ps[:, c_, 0:N],
                                        scalar=bres[:, 1, c_:c_ + 1], in1=h[:, c_, sl].bitcast(F32),
                                        op0=ALU.add, op1=ALU.add)
                            if l == D - 1:
                                # stream final outputs per sample to overlap the tail
                                red2 = temps.tile([128, 2], F32, tag="red2", bufs=2)
                                nc.vector.tensor_reduce(
                                    out=red2, in_=h[:, :, sl].bitcast(F32),
                                    op=ALU.add, axis=AX.X)
                                nc.vector.tensor_scalar_mul(mean_sb[:, :, s], red2, 1.0 / N)
                                for c_ in range(2):
                                    nc.sync.dma_start(
                                        out=d_oh[c_ * 128:(c_ + 1) * 128, s * N:(s + 1) * N],
                                        in_=h[:, c_, sl].bitcast(F32))

            # ---------------- outputs ----------------
            for c_ in range(2):
                nc.sync.dma_start(out=d_om[c_ * 128:(c_ + 1) * 128, :], in_=mean_sb[:, c_, :])

    nc.compile()
    return nc


def _host_prep(x, n_channel_offset, proj_W, proj_b, chan_tok, Wq, Wk, Wv, Wo, bo,
               ln1_g, ln1_b, ln2_g, ln2_b, W1, b1, W2, b2):
    f32 = np.float32
    x = np.ascontiguousarray(np.asarray(x, f32))
    Bx = x.shape[0]

    # im2col frames, transposed to (B, n_fft, N)
    sb, sc, st = x.strides
    fr = np.lib.stride_tricks.as_strided(
        x, shape=(Bx, C, NFR, N_FFT), strides=(sb, sc, HOP * st, st))
    frames = np.ascontiguousarray(fr.transpose(0, 3, 1, 2).reshape(Bx, N_FFT, N))

    # windowed DFT matrices
    n = np.arange(N_FFT, dtype=np.float64)
    win = 0.5 * (1.0 - np.cos(2.0 * np.pi * n / N_FFT))
    k = np.arange(NF, dtype=np.float64)
    ang = 2.0 * np.pi * np.outer(n, k) / N_FFT
    gmat = np.concatenate([win[:, None] * np.cos(ang),
                           -win[:, None] * np.sin(ang)], axis=1).astype(f32)

    # embedding bias: proj_b + chan_tok slice + positional encoding
    pos = np.arange(NFR, dtype=np.float64)[:, None]
    div = np.exp(np.arange(0, E, 2, dtype=np.float64) * (-np.log(10000.0) / E))
    pe = np.zeros((NFR, E), np.float64)
    pe[:, 0::2] = np.sin(pos * div)
    pe[:, 1::2] = np.cos(pos * div)
    off = int(n_channel_offset)
    ch = np.asarray(chan_tok, f32)[off:off + C]
    bias_all = (np.asarray(proj_b, f32)[None, None, :] + ch[:, None, :]
                + pe[None, :, :].astype(f32)).reshape(N, E)
    biasT = np.ascontiguousarray(bias_all.T)

    pp = lambda v: np.ascontiguousarray(np.asarray(v, f32).reshape(2, 128).T)
    lng = np.stack([np.stack([pp(ln1_g[l]), pp(ln2_g[l])]) for l in range(D)])
    lngn = -lng
    bo_pp = np.stack([pp(np.asarray(bo, f32)[l]) for l in range(D)])
    b2_pp = np.stack([pp(np.asarray(b2, f32)[l]) for l in range(D)])

    # biases that have no device path in this build must be zero
    assert not np.any(np.asarray(b1)), "nonzero b1 not supported by this build"
    assert not np.any(np.asarray(ln1_b)) and not np.any(np.asarray(ln2_b)), \
        "nonzero LN bias not supported by this build"

    blockones = np.zeros((128, 2, 8), f32)
    expander = np.zeros((8, 2, 128), f32)
    for p in range(128):
        blockones[p, 0, p // 32] = 1.0
        blockones[p, 1, 4 + p // 32] = 1.0
        expander[p // 32, 0, p] = 1.0
        expander[4 + p // 32, 1, p] = 1.0
    maskS = np.zeros((128, 128), f32)
    sc_ = 1.0 / np.sqrt(DH)
    for d_ in range(128):
        blk = d_ // 32
        maskS[d_, blk * 32:(blk + 1) * 32] = sc_

    g1v = np.asarray(ln1_g, f32)[:, :, None]   # (D, E, 1)
    g2v = np.asarray(ln2_g, f32)[:, :, None]
    com = dict(
        gmat=gmat, projw=np.asarray(proj_W, f32), biasT=biasT,
        wq=np.asarray(Wq, f32) * g1v, wk=np.asarray(Wk, f32) * g1v,
        wv=np.asarray(Wv, f32) * g1v,
        wo=np.asarray(Wo, f32), w1=np.asarray(W1, f32) * g2v, w2=np.asarray(W2, f32),
        lng=lng, lngn=lngn, bo=bo_pp, b2=b2_pp,
        blockones=blockones, expander=expander, maskS=maskS)
    return frames, com


def kernel(**inputs):
    from concourse.bass_utils import run_bass_kernel_spmd

    frames, com = _host_prep(**inputs)
    Bx = frames.shape[0]
    nsamp = Bx // NCORES
    sel = np.zeros((128, nsamp, nsamp), np.float32)
    for s in range(nsamp):
        sel[:, s, s] = 1.0
    com["sel"] = sel

    zb = (not np.any(np.asarray(inputs["bo"]))) and (not np.any(np.asarray(inputs["b2"])))
    key = (nsamp, zb)
    if key not in _CACHE:
        _CACHE[key] = _build(nsamp, zero_bres=zb)
    nc = _CACHE[key]

    in_maps = []
    for c in range(NCORES):
        m = dict(com)
        m["frames"] = np.ascontiguousarray(frames[c * nsamp:(c + 1) * nsamp])
        in_maps.append(m)
    res = run_bass_kernel_spmd(nc, in_maps, list(range(NCORES))).results

    h_out = np.empty((Bx, N, E), np.float32)
    mean_out = np.empty((Bx, E), np.float32)
    for c in range(NCORES):
        oh = res[c]["out_h"]                       # (E, nsamp*N)
        h_out[c * nsamp:(c + 1) * nsamp] = (
            oh.reshape(E, nsamp, N).transpose(1, 2, 0))
        mean_out[c * nsamp:(c + 1) * nsamp] = res[c]["out_mean"].T
    return mean_out, h_out
